# revision 1
# baseline (speedup 1.0000x reference)
"""LPCNet sampling kernel for Trainium2 — nn_LPCNet_91061896609827.

kernel(**inputs) takes FULL unsharded inputs (as from reference.setup_inputs())
and returns the FULL [B, F*T, K] float32 logits output.

Strategy: data-parallel over the R = B*F = 1024 row axis, 8 shards of 128 rows
(one per NeuronCore, rows on SBUF partitions), GRU/dense weights replicated.
The T=160 sequential sampling scan runs fully on-device per core:
  - all matmuls in fp32 on the PE (measured ~23-bit rms precision),
  - gate nonlinearities / exp / ln on the scalar engine LUTs,
  - round() via int32 cast (hardware round-half-even == jnp.round),
  - floor() via int32 cast of (v - 0.5),
  - softmax without max-subtraction (args bounded, fp32 safe).
Numerical discrepancy vs the jax fp32 reference stays at the 1e-5 level per
step, far below the ~1e-4/step level where discrete decision flips degrade
L2 beyond 2e-2 (the recurrence is contractive: flips wash out).

Self-contained: hardcodes shapes; reads nothing from /root/problem.
"""
import numpy as np
from contextlib import ExitStack

import concourse.bass as bass
import concourse.tile as tile
import concourse.mybir as mybir
from concourse import bacc
from concourse.bass_utils import run_bass_kernel_spmd

B, F, M, NF = 32, 32, 16, 20
T, K = 160, 256
R = B * F
COND, HA, HB = 128, 384, 16
N_CORES = 8
P = 128  # rows per core == SBUF partitions

f32 = mybir.dt.float32
bf16 = mybir.dt.bfloat16
i32 = mybir.dt.int32
AF = mybir.ActivationFunctionType
ALU = mybir.AluOpType

MU_C = np.float32(128.0 / np.log(np.float32(256.0)))  # 128/ln(256)


def _build(T_steps: int, dbg: bool = False):
    """Build the per-core Bass program (same program for all cores)."""
    nc = bacc.Bacc("TRN2", target_bir_lowering=False, debug=False,
                   num_devices=N_CORES)

    def din(name, shape):
        return nc.dram_tensor(name, list(shape), f32, kind="ExternalInput").ap()

    ins = {
        # per-core
        "feat": din("feat", [P, NF]),
        "lpcrot": din("lpcrot", [P, 16 * 16]),
        "u": din("u", [T_steps, P, K]),
        # replicated statics (host-packed)
        "wf1b": din("wf1b", [NF + 1, COND]),
        "wf2": din("wf2", [COND, COND]),
        "bf2": din("bf2", [1, COND]),
        "wxac": din("wxac", [COND, 3 * HA]),
        "ba": din("ba", [1, 3 * HA]),
        "wdynh": din("wdynh", [3, 3 * HA]),
        "wdynl": din("wdynl", [3, 3 * HA]),
        "wha": din("wha", [P, 3 * 3 * HA]),     # 3 K-chunks stacked on free axis
        "wxb": din("wxb", [P, 3 * 3 * HB]),     # 3 K-chunks of [128, 48]
        "bb": din("bb", [1, 3 * HB]),
        "whb": din("whb", [HB, 3 * HB]),
        "w12b": din("w12b", [HB + 1, 2 * K]),   # [W1|W2 ; b1|b2]
        "g12": din("g12", [P, 2 * K]),          # [g1|g2] row broadcast
        "idxb": din("idxb", [P, K]),            # iota row broadcast
        "ident": din("ident", [P, P]),
        "onesrow": din("onesrow", [1, P]),
        "taub": din("taub", [P, K]),
    }
    out_l = nc.dram_tensor("logits", [T_steps, P, K], f32, kind="ExternalOutput").ap()
    g_dram = nc.dram_tensor("g_scratch", [T_steps, P, K], f32).ap()
    dbg_outs = {}
    if dbg:
        for nm, sh in [("d_cond", [P, COND]), ("d_gxs", [P, 3 * HA]),
                       ("d_rzt", [P, 768]), ("d_rz", [P, 768]), ("d_an", [P, HA]),
                       ("d_ha", [P, HA]), ("d_hb", [P, HB]), ("d_ghbs", [P, 3 * HB]),
                       ("d_gxb", [P, 3 * HB]), ("d_l12", [P, 2 * K]),
                       ("d_soft", [P, 1]), ("d_p", [P, 1]),
                       ("d_pdot", [P, 1]), ("d_psgn", [P, 1]), ("d_pln", [P, 1]),
                       ("d_pv", [P, 1]), ("d_pvh", [P, 1]), ("d_pflt", [P, 1])]:
            dbg_outs[nm] = nc.dram_tensor(nm, sh, f32, kind="ExternalOutput").ap()

    G3 = 3 * HA  # 1152

    with tile.TileContext(nc) as tc, ExitStack() as ctx:
        st = ctx.enter_context(tc.tile_pool(name="static", bufs=1))
        wk = ctx.enter_context(tc.tile_pool(name="work", bufs=2))
        io = ctx.enter_context(tc.tile_pool(name="io", bufs=3))
        ps_rz = ctx.enter_context(tc.tile_pool(name="ps_rz", bufs=1, space="PSUM"))
        ps_n = ctx.enter_context(tc.tile_pool(name="ps_n", bufs=1, space="PSUM"))
        ps_x = ctx.enter_context(tc.tile_pool(name="ps_x", bufs=1, space="PSUM"))
        ps_b = ctx.enter_context(tc.tile_pool(name="ps_b", bufs=1, space="PSUM"))
        ps_h = ctx.enter_context(tc.tile_pool(name="ps_h", bufs=1, space="PSUM"))
        ps_l = ctx.enter_context(tc.tile_pool(name="ps_l", bufs=1, space="PSUM"))
        ps_t = ctx.enter_context(tc.tile_pool(name="ps_t", bufs=1, space="PSUM"))

        # ---- load statics ----
        def load(name, shape, tag=None):
            t_ = st.tile(list(shape), f32, tag=tag or name)
            nc.sync.dma_start(t_[:], ins[name][:])
            return t_

        feat = load("feat", [P, NF])
        lpcrot = load("lpcrot", [P, 256])
        wf1b = load("wf1b", [NF + 1, COND])
        wf2 = load("wf2", [COND, COND])
        bf2 = load("bf2", [1, COND])
        wxac = load("wxac", [COND, G3])
        ba = load("ba", [1, G3])
        wdynhf = load("wdynh", [3, G3])
        wdynlf = load("wdynl", [3, G3])
        wdynh = st.tile([3, G3], bf16, tag="wdynh_b")
        wdynl = st.tile([3, G3], bf16, tag="wdynl_b")
        nc.vector.tensor_copy(wdynh[:], wdynhf[:])
        nc.vector.tensor_copy(wdynl[:], wdynlf[:])
        wha = load("wha", [P, 3 * G3])
        wxb = load("wxb", [P, 3 * 3 * HB])
        bb = load("bb", [1, 3 * HB])
        whb = load("whb", [HB, 3 * HB])
        w12b = load("w12b", [HB + 1, 2 * K])
        g12 = load("g12", [P, 2 * K])
        idxb = load("idxb", [P, K])
        ident = load("ident", [P, P])
        taub = load("taub", [P, K])
        identb = st.tile([P, P], bf16, tag="identb")
        nc.vector.tensor_copy(identb[:], ident[:])

        # ---- persistent state ----
        haTw = st.tile([P, 3 * P], f32, tag="haTw")
        haT = [haTw[:, c * P:(c + 1) * P] for c in range(3)]
        hbTa = st.tile([HB + 1, P], f32, tag="hbTa")
        ring = st.tile([P, 16], f32, tag="ring")
        dynstage = st.tile([P, 3], bf16, tag="dynstage")
        ones1 = st.tile([1, P], f32, tag="ones1")
        gxs = st.tile([P, G3], f32, tag="gxs")

        nc.vector.memset(haTw[:], 0.0)
        nc.vector.memset(hbTa[:HB, :], 0.0)
        nc.sync.dma_start(hbTa[HB:, :], ins["onesrow"][:])
        nc.vector.memset(ring[:], 0.0)
        nc.vector.memset(dynstage[:], 0.0)
        nc.vector.memset(ones1[:], 1.0)

        # ---- conditioning network (one-time) ----
        tp = ps_t.tile([P, P], f32, tag="tpw")
        nc.tensor.transpose(tp[:NF, :], feat[:], ident[:])
        featTa = st.tile([NF + 1, P], f32, tag="featTa")
        nc.scalar.copy(featTa[:NF, :], tp[:NF, :])
        nc.sync.dma_start(featTa[NF:, :], ins["onesrow"][:])

        h1ps = ps_l.tile([P, COND], f32, tag="lps")
        nc.tensor.matmul(h1ps[:], featTa[:], wf1b[:], start=True, stop=True)
        h1 = wk.tile([P, COND], f32, tag="h1")
        nc.scalar.activation(h1[:], h1ps[:], AF.Tanh)

        tp = ps_t.tile([P, P], f32, tag="tpw")
        nc.tensor.transpose(tp[:], h1[:], ident[:])
        h1T = wk.tile([P, P], f32, tag="h1T")
        nc.scalar.copy(h1T[:], tp[:])

        cps = ps_l.tile([P, COND], f32, tag="lps")
        nc.tensor.matmul(cps[:], h1T[:], wf2[:], start=True, stop=False)
        nc.tensor.matmul(cps[:], ones1[:], bf2[:], start=False, stop=True)
        cond = wk.tile([P, COND], f32, tag="h1")
        nc.scalar.activation(cond[:], cps[:], AF.Tanh)
        if dbg:
            nc.sync.dma_start(dbg_outs["d_cond"][:], cond[:])

        tp = ps_t.tile([P, P], f32, tag="tpw")
        nc.tensor.transpose(tp[:], cond[:], ident[:])
        condT = wk.tile([P, P], f32, tag="h1T")
        nc.scalar.copy(condT[:], tp[:])

        # gx_static = cond @ Wxa[:COND] + ba  -> [P, 1152]
        for sl in ((0, 512), (512, 1024), (1024, G3)):
            gsps = ps_rz.tile([P, 768], f32, tag="garz")
            nc.tensor.matmul(gsps[:, :sl[1] - sl[0]], condT[:], wxac[:, sl[0]:sl[1]],
                             start=True, stop=False)
            nc.tensor.matmul(gsps[:, :sl[1] - sl[0]], ones1[:], ba[:, sl[0]:sl[1]],
                             start=False, stop=True)
            nc.vector.tensor_copy(gxs[:, sl[0]:sl[1]], gsps[:, :sl[1] - sl[0]])
        if dbg:
            nc.sync.dma_start(dbg_outs["d_gxs"][:], gxs[:])

        # ---- gumbel pre-pass: g_dram[t] = ln(-ln(u[t])) ----
        GB = 16
        for cch in range((T_steps + GB - 1) // GB):
            nsteps = min(GB, T_steps - cch * GB)
            gt = wk.tile([P, GB * K], f32, tag="gpre")
            src_ap = ins["u"][cch * GB:cch * GB + nsteps, :, :].rearrange(
                "t p k -> p t k")
            nc.sync.dma_start(gt[:, :nsteps * K].rearrange(
                "p (t k) -> p t k", t=nsteps), src_ap)
            ga1 = wk.tile([P, GB * K], f32, tag="gpre2")
            nc.scalar.activation(ga1[:, :nsteps * K], gt[:, :nsteps * K], AF.Ln)
            nc.scalar.activation(gt[:, :nsteps * K], ga1[:, :nsteps * K],
                                 AF.Ln, scale=-1.0)
            dst_ap = g_dram[cch * GB:cch * GB + nsteps, :, :].rearrange(
                "t p k -> p t k")
            nc.sync.dma_start(dst_ap, gt[:, :nsteps * K].rearrange(
                "p (t k) -> p t k", t=nsteps))

        # ---- time loop ----
        for t in range(T_steps):
            # p path: dot = sum(lpc_rot * ring); p = #{tau_k <= dot}
            rot = (t % 16) * 16
            sc16 = wk.tile([P, 16], f32, tag="sc16")
            pdot = wk.tile([P, 1], f32, tag="pdot")
            nc.vector.scalar_tensor_tensor(sc16[:], lpcrot[:, rot:rot + 16], 0.0,
                                           ring[:], op0=ALU.bypass, op1=ALU.mult,
                                           accum_out=pdot[:])
            pscr = wk.tile([P, K], f32, tag="pscr")
            nc.vector.tensor_scalar(pscr[:], taub[:], pdot[:], 0.0,
                                    op0=ALU.is_le, op1=ALU.add,
                                    accum_out=dynstage[:, 0:1])
            # transpose dyn stage -> [3, P]
            tdps = ps_t.tile([P, P], bf16, tag="tpw")
            nc.tensor.transpose(tdps[:3, :], dynstage[:], identb[:])
            dynT = wk.tile([3, P], bf16, tag="dynT")
            nc.scalar.copy(dynT[:], tdps[:3, :])

            # GRU-A matmuls
            garz = ps_rz.tile([P, 768], f32, tag="garz")
            ghn = ps_n.tile([P, HA], f32, tag="ghn")
            gxn = ps_x.tile([P, HA], f32, tag="gxn")
            nc.tensor.matmul(garz[:, 0:512], dynT[:], wdynh[:, 0:512],
                             start=True, stop=False)
            nc.tensor.matmul(garz[:, 0:512], dynT[:], wdynl[:, 0:512],
                             start=False, stop=False)
            nc.tensor.matmul(garz[:, 512:768], dynT[:], wdynh[:, 512:768],
                             start=True, stop=False)
            nc.tensor.matmul(garz[:, 512:768], dynT[:], wdynl[:, 512:768],
                             start=False, stop=False)
            nc.tensor.matmul(gxn[:], dynT[:], wdynh[:, 768:G3], start=True, stop=False)
            nc.tensor.matmul(gxn[:], dynT[:], wdynl[:, 768:G3], start=False, stop=True)
            for c in range(3):
                w0 = c * G3
                nc.tensor.matmul(garz[:, 0:512], haT[c], wha[:, w0:w0 + 512],
                                 start=False, stop=(c == 2))
                nc.tensor.matmul(garz[:, 512:768], haT[c], wha[:, w0 + 512:w0 + 768],
                                 start=False, stop=(c == 2))
                nc.tensor.matmul(ghn[:], haT[c], wha[:, w0 + 768:w0 + G3],
                                 start=(c == 0), stop=(c == 2))

            # gates
            rzt = wk.tile([P, 768], f32, tag="rzt")
            nc.vector.scalar_tensor_tensor(rzt[:], garz[:], 0.0, gxs[:, 0:768],
                                           op0=ALU.bypass, op1=ALU.add)
            tzh = wk.tile([P, 768], f32, tag="tzh")
            nc.scalar.activation(tzh[:], rzt[:], AF.Tanh, scale=0.5)
            rz = wk.tile([P, 768], f32, tag="rz")
            nc.vector.tensor_scalar(rz[:], tzh[:], 0.5, 0.5, op0=ALU.mult, op1=ALU.add)
            t1 = wk.tile([P, HA], f32, tag="t1")
            nc.vector.tensor_tensor(t1[:], rz[:, 0:HA], ghn[:], op=ALU.mult)
            t2 = wk.tile([P, HA], f32, tag="t2")
            nc.vector.scalar_tensor_tensor(t2[:], gxn[:], 0.0, gxs[:, 768:G3],
                                           op0=ALU.bypass, op1=ALU.add)
            t3 = wk.tile([P, HA], f32, tag="t3")
            nc.vector.tensor_tensor(t3[:], t1[:], t2[:], op=ALU.add)
            an = wk.tile([P, HA], f32, tag="an")
            nc.scalar.activation(an[:], t3[:], AF.Tanh)
            if dbg and t == 0:
                nc.sync.dma_start(dbg_outs["d_rzt"][:], rzt[:])
                nc.sync.dma_start(dbg_outs["d_rz"][:], rz[:])
                nc.sync.dma_start(dbg_outs["d_an"][:], an[:])

            # ha update: ha2 = n + z*(ha - n)
            ha_prev = ha_rm if t > 0 else None
            if t == 0:
                ha_rm = wk.tile([P, HA], f32, tag="ha_rm")
                nc.vector.memset(ha_rm[:], 0.0)
                ha_prev = ha_rm
            d = wk.tile([P, HA], f32, tag="d")
            nc.vector.tensor_tensor(d[:], ha_prev[:], an[:], op=ALU.subtract)
            zd = wk.tile([P, HA], f32, tag="zd")
            nc.vector.tensor_tensor(zd[:], d[:], rz[:, HA:768], op=ALU.mult)
            ha_rm = wk.tile([P, HA], f32, tag="ha_rm")
            nc.vector.tensor_tensor(ha_rm[:], zd[:], an[:], op=ALU.add)
            if dbg and t == 0:
                nc.sync.dma_start(dbg_outs["d_ha"][:], ha_rm[:])

            # transpose ha2 -> haT chunks (one psum bank, one copy)
            tpc = ps_t.tile([P, 3 * P], f32, tag="tpw")
            for c in range(3):
                nc.tensor.transpose(tpc[:, c * P:(c + 1) * P],
                                    ha_rm[:, c * P:(c + 1) * P], ident[:])
            nc.scalar.copy(haTw[:], tpc[:])

            # GRU-B
            gxb = ps_b.tile([P, 3 * HB], f32, tag="gxb")
            for c in range(3):
                nc.tensor.matmul(gxb[:], haT[c], wxb[:, c * 48:(c + 1) * 48],
                                 start=(c == 0), stop=False)
            nc.tensor.matmul(gxb[:], ones1[:], bb[:], start=False, stop=True)
            ghb = ps_h.tile([P, 3 * HB], f32, tag="ghb")
            nc.tensor.matmul(ghb[:], hbTa[:HB, :], whb[:], start=True, stop=True)
            ghbs = wk.tile([P, 3 * HB], f32, tag="ghbs")
            nc.scalar.copy(ghbs[:], ghb[:])

            rzbt = wk.tile([P, 2 * HB], f32, tag="rzbt")
            nc.vector.scalar_tensor_tensor(rzbt[:], gxb[:, 0:2 * HB], 0.0,
                                           ghbs[:, 0:2 * HB],
                                           op0=ALU.bypass, op1=ALU.add)
            tzb = wk.tile([P, 2 * HB], f32, tag="tzb")
            nc.scalar.activation(tzb[:], rzbt[:], AF.Tanh, scale=0.5)
            rzb = wk.tile([P, 2 * HB], f32, tag="rzb")
            nc.vector.tensor_scalar(rzb[:], tzb[:], 0.5, 0.5, op0=ALU.mult, op1=ALU.add)
            t1b = wk.tile([P, HB], f32, tag="t1b")
            nc.vector.tensor_tensor(t1b[:], rzb[:, 0:HB], ghbs[:, 2 * HB:3 * HB],
                                    op=ALU.mult)
            t2b = wk.tile([P, HB], f32, tag="t2b")
            nc.vector.scalar_tensor_tensor(t2b[:], gxb[:, 2 * HB:3 * HB], 0.0,
                                           t1b[:], op0=ALU.bypass, op1=ALU.add)
            nb = wk.tile([P, HB], f32, tag="nb")
            nc.scalar.activation(nb[:], t2b[:], AF.Tanh)

            hb_prev = hb_rm if t > 0 else None
            if t == 0:
                hb_rm = wk.tile([P, HB], f32, tag="hb_rm")
                nc.vector.memset(hb_rm[:], 0.0)
                hb_prev = hb_rm
            db = wk.tile([P, HB], f32, tag="db")
            nc.vector.tensor_tensor(db[:], hb_prev[:], nb[:], op=ALU.subtract)
            zdb = wk.tile([P, HB], f32, tag="zdb")
            nc.vector.tensor_tensor(zdb[:], db[:], rzb[:, HB:2 * HB], op=ALU.mult)
            hb_rm = wk.tile([P, HB], f32, tag="hb_rm")
            nc.vector.tensor_tensor(hb_rm[:], zdb[:], nb[:], op=ALU.add)
            if dbg and t == 0:
                nc.sync.dma_start(dbg_outs["d_hb"][:], hb_rm[:])
                nc.sync.dma_start(dbg_outs["d_ghbs"][:], ghbs[:])
                gxbs_d = wk.tile([P, 3 * HB], f32, tag="gxbs_d")
                nc.vector.tensor_copy(gxbs_d[:], gxb[:])
                nc.sync.dma_start(dbg_outs["d_gxb"][:], gxbs_d[:])

            tpb = ps_t.tile([P, P], f32, tag="tpw")
            nc.tensor.transpose(tpb[:HB, :], hb_rm[:], ident[:])
            nc.scalar.copy(hbTa[:HB, :], tpb[:HB, :])

            # logits
            lps = ps_l.tile([P, 2 * K], f32, tag="lps")
            nc.tensor.matmul(lps[:], hbTa[:], w12b[:], start=True, stop=True)
            l12 = wk.tile([P, 2 * K], f32, tag="l12")
            nc.scalar.activation(l12[:], lps[:], AF.Tanh)
            if dbg and t == 0:
                nc.sync.dma_start(dbg_outs["d_l12"][:], l12[:])
            lg12 = wk.tile([P, 2 * K], f32, tag="lg12")
            nc.vector.tensor_tensor(lg12[:], l12[:], g12[:], op=ALU.mult)
            logits = io.tile([P, K], f32, tag="logits")
            nc.vector.tensor_tensor(logits[:], lg12[:, 0:K], lg12[:, K:2 * K],
                                    op=ALU.add)
            nc.sync.dma_start(out_l[t, :, :], logits[:])

            # gumbel softmax expected index
            a2 = io.tile([P, K], f32, tag="a2")
            nc.sync.dma_start(a2[:], g_dram[t, :, :])
            zz = wk.tile([P, K], f32, tag="zz")
            nc.vector.tensor_tensor(zz[:], logits[:], a2[:], op=ALU.subtract)
            E = wk.tile([P, K], f32, tag="E")
            den = wk.tile([P, 1], f32, tag="den")
            nc.scalar.activation(E[:], zz[:], AF.Exp, accum_out=den[:])
            Escr = wk.tile([P, K], f32, tag="Escr")
            num = wk.tile([P, 1], f32, tag="num")
            nc.vector.scalar_tensor_tensor(Escr[:], E[:], 0.0, idxb[:],
                                           op0=ALU.bypass, op1=ALU.mult,
                                           accum_out=num[:])
            rden = wk.tile([P, 1], f32, tag="rden")
            nc.vector.reciprocal(rden[:], den[:])
            soft = wk.tile([P, 1], f32, tag="soft")
            nc.vector.tensor_tensor(soft[:], num[:], rden[:], op=ALU.mult)
            if dbg and t == 0:
                nc.sync.dma_start(dbg_outs["d_soft"][:], soft[:])
                nc.sync.dma_start(dbg_outs["d_p"][:], dynstage[:, 0:1])
            sofh = wk.tile([P, 1], f32, tag="sofh")
            nc.vector.tensor_scalar(sofh[:], soft[:], 0.5, None, op0=ALU.add)
            eint = wk.tile([P, 1], i32, tag="eint")
            nc.vector.tensor_copy(eint[:], sofh[:])
            eflt = wk.tile([P, 1], f32, tag="eflt")
            nc.vector.tensor_copy(eflt[:], eint[:])
            emask = wk.tile([P, 1], f32, tag="emask")
            nc.vector.tensor_tensor(emask[:], eflt[:], sofh[:], op=ALU.is_gt)
            # e -> dynstage col2 (e_prev for next step)
            nc.vector.tensor_tensor(dynstage[:, 2:3], eflt[:], emask[:],
                                    op=ALU.subtract)
            # s = p + e -> ring slot and dynstage col1
            nc.vector.tensor_tensor(ring[:, t % 16:t % 16 + 1], dynstage[:, 0:1],
                                    dynstage[:, 2:3], op=ALU.add)
            nc.vector.tensor_copy(dynstage[:, 1:2], dynstage[:, 0:1])

    return nc, ins, out_l


def _pack_inputs(frames_features, lpc_coeffs, gumbel_u, Wf1, bf1, Wf2, bf2,
                 Wxa, Wha, ba, Wxb, Whb, bb, W1, b1, g1, W2, b2, g2,
                 T_steps=T):
    """Host-side packing -> (statics dict, per-core dicts list)."""
    fp = np.float32
    feat = np.ascontiguousarray(frames_features, fp).reshape(R, NF)
    lpc = np.ascontiguousarray(lpc_coeffs, fp).reshape(R, M)
    u = np.ascontiguousarray(gumbel_u, fp)

    # lpcrot[:, 16*r + j] = lpc[:, (j - r) % 16]
    lpcrot = np.empty((R, 256), fp)
    for r_ in range(16):
        for j in range(16):
            lpcrot[:, 16 * r_ + j] = lpc[:, (j - r_) % 16]

    wha_p = np.concatenate([np.ascontiguousarray(Wha[c * P:(c + 1) * P, :], fp)
                            for c in range(3)], axis=1)          # [128, 3*1152]
    wxb_p = np.concatenate([np.ascontiguousarray(Wxb[c * P:(c + 1) * P, :], fp)
                            for c in range(3)], axis=1)          # [128, 144]
    statics = {
        "wf1b": np.concatenate([np.asarray(Wf1, fp), np.asarray(bf1, fp)[None, :]], 0),
        "wf2": np.asarray(Wf2, fp),
        "bf2": np.asarray(bf2, fp)[None, :],
        "wxac": np.ascontiguousarray(np.asarray(Wxa, fp)[:COND, :]),
        "ba": np.asarray(ba, fp)[None, :],
        "wdynh": _wdyn2(np.asarray(Wxa, fp))[0:3],
        "wdynl": _wdyn2(np.asarray(Wxa, fp))[3:6],
        "wha": wha_p,
        "wxb": wxb_p,
        "bb": np.asarray(bb, fp)[None, :],
        "whb": np.asarray(Whb, fp),
        "w12b": np.concatenate([
            np.concatenate([np.asarray(W1, fp), np.asarray(W2, fp)], axis=1),
            np.concatenate([np.asarray(b1, fp), np.asarray(b2, fp)])[None, :]], 0),
        "g12": np.repeat(np.concatenate([np.asarray(g1, fp), np.asarray(g2, fp)])[None, :], P, 0),
        "idxb": np.repeat(np.arange(K, dtype=fp)[None, :], P, 0),
        "ident": np.eye(P, dtype=fp),
        "onesrow": np.ones((1, P), fp),
        "taub": np.repeat(_tau_table()[None, :], P, 0),
    }
    per_core = []
    for c in range(N_CORES):
        rs = slice(c * P, (c + 1) * P)
        m = dict(statics)
        m["feat"] = np.ascontiguousarray(feat[rs])
        m["lpcrot"] = np.ascontiguousarray(lpcrot[rs])
        m["u"] = np.ascontiguousarray(u[:T_steps, rs, :])
        per_core.append(m)
    return per_core


def _wdyn2(Wxa):
    """[6, 3H]: bf16 hi/lo split of [w_p, w_s, w_s+w_e] (exact-bf16 dyn matmul)."""
    import ml_dtypes
    fp = np.float32
    wd = Wxa[COND:COND + 3, :].astype(fp).copy()
    wd[2] = (wd[1] + wd[2]).astype(fp)
    hi = wd.astype(ml_dtypes.bfloat16).astype(fp)
    lo = (wd - hi).astype(ml_dtypes.bfloat16).astype(fp)
    return np.concatenate([hi, lo], 0)


def _tau_table():
    """tau[k] = smallest float32 x with mu_law_p(x) >= k+1 (k=0..254);
    tau[255] = +inf sentinel. p(x) = sum_k [x >= tau_k]."""
    fp = np.float32

    def p_of(x):
        x = np.asarray(x, fp)
        xc = np.clip(x, fp(-1.0), fp(1.0)).astype(fp)
        ln_mu1 = np.log(fp(256.0)).astype(fp)
        y = (np.sign(xc) * np.log1p(fp(255.0) * np.abs(xc)) / ln_mu1).astype(fp)
        v = ((y + fp(1.0)) * fp(0.5) * fp(256.0)).astype(fp)
        return np.clip(np.floor(v), 0.0, 255.0)

    def f2i(x):
        b = np.asarray(x, np.float32).view(np.int32)
        return np.where(b < 0, np.int32(-2147483648) - b, b).astype(np.int64)

    def i2f(i):
        i = np.asarray(i, np.int64)
        b = np.where(i < 0, -2147483648 - i, i).astype(np.int32)
        return b.view(np.float32)

    ks = np.arange(1, 256)
    lo = np.full(255, f2i(np.float32(-1.5)), np.int64)
    hi = np.full(255, f2i(np.float32(1.5)), np.int64)
    for _ in range(40):
        mid = (lo + hi) // 2
        ge = p_of(i2f(mid)) >= ks
        hi = np.where(ge, mid, hi)
        lo = np.where(ge, lo, mid)
    tau = i2f(hi).astype(fp)
    out = np.empty(256, fp)
    out[:255] = tau
    out[255] = np.float32(3.0e38)
    return out


_CACHE = {}


def _ensure_devices():
    """Make sure the 8 NeuronCores are visible to jax (the PJRT execute
    path uses jax.devices()), even if the host process pinned cpu."""
    import jax
    try:
        if len(jax.devices()) >= N_CORES:
            return
    except Exception:
        pass
    jax.config.update("jax_platforms", "axon,cpu")
    import jax.extend.backend as _jeb
    _jeb.clear_backends()
    assert len(jax.devices()) >= N_CORES, (
        f"need {N_CORES} NeuronCores, visible: {jax.devices()}")


def _get_nc(T_steps):
    if T_steps not in _CACHE:
        nc, ins, out_l = _build(T_steps)
        nc.compile()
        _CACHE[T_steps] = nc
    return _CACHE[T_steps]


def kernel(**inputs):
    _ensure_devices()
    nc = _get_nc(T)
    per_core = _pack_inputs(**inputs)
    res = run_bass_kernel_spmd(nc, per_core, list(range(N_CORES)))
    shards = [res.results[c]["logits"] for c in range(N_CORES)]   # each [T,128,K]
    logits_seq = np.concatenate(shards, axis=1)                   # [T, R, K]
    out = logits_seq.transpose(1, 0, 2).reshape(B, F * T, K)
    return np.ascontiguousarray(out, dtype=np.float32)



# revision 22
# speedup vs baseline: 1.6508x; 1.6508x over previous
"""LPCNet sampling kernel for Trainium2 — nn_LPCNet_91061896609827.

kernel(**inputs) takes FULL unsharded inputs (as from reference.setup_inputs())
and returns the FULL [B, F*T, K] float32 logits output.

Strategy: data-parallel over the R = B*F = 1024 row axis, 8 shards of 128 rows
(one per NeuronCore, rows on SBUF partitions), GRU/dense weights replicated.
The T=160 sequential sampling scan runs fully on-device per core.

v2 vs v1 (5.01ms):
  - gumbel g = ln(-ln u) precomputed on HOST; no device prepass, no u DMA.
  - all recurrent matmuls single-pass bf16 (fp32 matmul = 2 PE passes);
    dyn (p/s/e feature) matmul keeps exact bf16 hi/lo split.
  - static gate biases (cond@Wxa+ba, bb) folded into PSUM accumulation via
    identity/ones matmuls -> sigmoid/tanh read PSUM directly, the wide
    vector adds disappear from the serial chain.
  - garz accumulation reordered: the big wha matmuls for step t+1 are
    emitted at the end of step t (hidden under GRU-B/sampling); only the
    tiny dyn matmuls sit on the critical path.
  - round(soft) via direct f32->i32 cast (hardware round-half-even ==
    jnp.round; verified by probe).
  - sigmoid LUT (one op) instead of tanh(0.5x) rescaling (three ops);
    r/z sigmoids split so r is ready earlier.
  - ha transpose copies spread across scalar/vector/gpsimd engines.

Self-contained: hardcodes shapes; reads nothing from /root/problem.
"""
import numpy as np
from contextlib import ExitStack

import concourse.bass as bass
import concourse.tile as tile
import concourse.mybir as mybir
from concourse import bacc
from concourse.bass_utils import run_bass_kernel_spmd

B, F, M, NF = 32, 32, 16, 20
T, K = 160, 256
R = B * F
COND, HA, HB = 128, 384, 16
N_CORES = 8
P = 128  # rows per core == SBUF partitions

f32 = mybir.dt.float32
bf16 = mybir.dt.bfloat16
i32 = mybir.dt.int32
f32r = mybir.dt.float32r
AF = mybir.ActivationFunctionType
ALU = mybir.AluOpType

G3 = 3 * HA  # 1152


def _build(T_steps: int, dbg: bool = False):
    nc = bacc.Bacc("TRN2", target_bir_lowering=False, debug=False,
                   num_devices=N_CORES)

    def din(name, shape, dt=f32):
        return nc.dram_tensor(name, list(shape), dt, kind="ExternalInput").ap()

    dbg_outs = {}
    if dbg:
        for nm, sh in [("d_soft0", [P, 1]), ("d_e0", [P, 1]), ("d_s0", [P, 1]),
                       ("d_p0", [P, 1]), ("d_p1", [P, 1]), ("d_pdot1", [P, 1]),
                       ("d_ha0", [P, HA]), ("d_hb0", [P, HB]),
                       ("d_garz1", [P, 768]), ("d_ghn1", [P, HA]),
                       ("d_gxn1", [P, HA]), ("d_r1", [P, HA]),
                       ("d_an1", [P, HA]), ("d_ha1", [P, HA]),
                       ("d_gb1", [P, 64]), ("d_hb1", [P, HB]),
                       ("d_dynT1", [3, P]), ("d_hbTa1", [HB + 1, P]),
                       ("d_gb1o", [P, 64])]:
            dbg_outs[nm] = nc.dram_tensor(nm, sh, f32, kind="ExternalOutput").ap()

    ins = {
        # per-core
        "feat": din("feat", [P, NF]),
        "lpcrot": din("lpcrot", [P, 16 * 16]),
        "gl": din("gl", [T_steps, P, K]),          # ln(-ln u), host-computed
        # replicated statics (host-packed)
        "wf1b": din("wf1b", [NF + 1, COND]),
        "wf2": din("wf2", [COND, COND]),
        "bf2": din("bf2", [1, COND]),
        "wxac": din("wxac", [COND, G3]),
        "ba": din("ba", [1, G3]),
        "wdynh": din("wdynh", [3, G3], bf16),
        "wdynl": din("wdynl", [3, G3], bf16),
        "wha": din("wha", [P, 3 * G3], f32r),      # 3 K-chunks on free axis
        "wxb": din("wxb", [P, 3 * 48], f32r),      # chunk c: [rz 32 | n 16]
        "whbbx": din("whbbx", [HB + 1, 64], f32r),  # [[Whb_rz;bb_rz] | [0;bb_n] | [Whb_n;0]]
        "w12b": din("w12b", [HB + 1, 2 * K], f32r),  # [W1|W2 ; b1|b2]
        "g12": din("g12", [P, 2 * K]),             # [g1|g2] row broadcast
        "idxb": din("idxb", [P, K]),               # iota row broadcast
        "ident": din("ident", [P, P]),
        "identr": din("identr", [P, P], f32r),
        "onesrow": din("onesrow", [1, P]),
        "onesrowr": din("onesrowr", [1, P], f32r),
        "zhaT": din("zhaT", [P, 3 * P], f32r),
        "zhbT": din("zhbT", [HB, P], f32r),
        "taub": din("taub", [P, K]),
    }
    out_l = nc.dram_tensor("logits", [T_steps, P, K], f32,
                           kind="ExternalOutput").ap()

    with tile.TileContext(nc) as tc, ExitStack() as ctx:
        st = ctx.enter_context(tc.tile_pool(name="static", bufs=1))
        wk = ctx.enter_context(tc.tile_pool(name="work", bufs=2))
        io = ctx.enter_context(tc.tile_pool(name="io", bufs=3))
        ps_rz = ctx.enter_context(tc.tile_pool(name="ps_rz", bufs=1, space="PSUM"))
        ps_n = ctx.enter_context(tc.tile_pool(name="ps_n", bufs=1, space="PSUM"))
        ps_x = ctx.enter_context(tc.tile_pool(name="ps_x", bufs=1, space="PSUM"))
        ps_b = ctx.enter_context(tc.tile_pool(name="ps_b", bufs=1, space="PSUM"))
        ps_l = ctx.enter_context(tc.tile_pool(name="ps_l", bufs=1, space="PSUM"))
        ps_t = ctx.enter_context(tc.tile_pool(name="ps_t", bufs=1, space="PSUM"))

        # ---- load statics ----
        def load(name, shape, dt=f32, tag=None):
            t_ = st.tile(list(shape), dt, tag=tag or name)
            nc.sync.dma_start(t_[:], ins[name][:])
            return t_

        feat = load("feat", [P, NF])
        lpcrot = load("lpcrot", [P, 256])
        wf1b = load("wf1b", [NF + 1, COND])
        wf2 = load("wf2", [COND, COND])
        bf2 = load("bf2", [1, COND])
        wxac = load("wxac", [COND, G3])
        ba = load("ba", [1, G3])
        wdynh = load("wdynh", [3, G3], bf16)
        wdynl = load("wdynl", [3, G3], bf16)
        wha = load("wha", [P, 3 * G3], f32r)
        wxb = load("wxb", [P, 3 * 48], f32r)
        whbbx = load("whbbx", [HB + 1, 64], f32r)
        w12b = load("w12b", [HB + 1, 2 * K], f32r)
        g12 = load("g12", [P, 2 * K])
        idxb = load("idxb", [P, K])
        ident = load("ident", [P, P])
        identr = load("identr", [P, P], f32r)
        taub = load("taub", [P, K])

        # ---- persistent state ----
        haTw = st.tile([P, 3 * P], f32r, tag="haTw")
        haT = [haTw[:, c * P:(c + 1) * P] for c in range(3)]
        hbTa = st.tile([HB + 1, P], f32r, tag="hbTa")
        ring = st.tile([P, 16], f32, tag="ring")
        dynstage = st.tile([P, 3], f32, tag="dynstage")
        gxs = st.tile([P, G3], f32, tag="gxs")
        gxsr = st.tile([P, G3], f32r, tag="gxsr")

        nc.sync.dma_start(haTw[:], ins["zhaT"][:])
        nc.sync.dma_start(hbTa[:HB, :], ins["zhbT"][:])
        nc.sync.dma_start(hbTa[HB:, :], ins["onesrowr"][:])
        nc.vector.memset(ring[:], 0.0)
        nc.vector.memset(dynstage[:], 0.0)

        # ---- conditioning network (one-time) ----
        ones1 = st.tile([1, P], f32, tag="ones1")
        nc.vector.memset(ones1[:], 1.0)
        tp = ps_t.tile([P, 512], f32, tag="tpw")
        nc.tensor.transpose(tp[:NF, 0:P], feat[:], ident[:])
        featTa = st.tile([NF + 1, P], f32, tag="featTa")
        nc.scalar.copy(featTa[:NF, :], tp[:NF, 0:P])
        nc.sync.dma_start(featTa[NF:, :], ins["onesrow"][:])

        h1ps = ps_l.tile([P, 2 * K], f32, tag="lps")
        nc.tensor.matmul(h1ps[:, :COND], featTa[:], wf1b[:], start=True, stop=True)
        h1 = wk.tile([P, COND], f32, tag="h1")
        nc.scalar.activation(h1[:], h1ps[:, :COND], AF.Tanh)

        tp = ps_t.tile([P, 512], f32, tag="tpw")
        nc.tensor.transpose(tp[:, 0:P], h1[:], ident[:])
        h1T = wk.tile([P, P], f32, tag="h1T")
        nc.scalar.copy(h1T[:], tp[:, 0:P])

        cps = ps_l.tile([P, 2 * K], f32, tag="lps")
        nc.tensor.matmul(cps[:, :COND], h1T[:], wf2[:], start=True, stop=False)
        nc.tensor.matmul(cps[:, :COND], ones1[:], bf2[:], start=False, stop=True)
        cond = wk.tile([P, COND], f32, tag="h1")
        nc.scalar.activation(cond[:], cps[:, :COND], AF.Tanh)

        tp = ps_t.tile([P, 512], f32, tag="tpw")
        nc.tensor.transpose(tp[:, 0:P], cond[:], ident[:])
        condT = wk.tile([P, P], f32, tag="h1T")
        nc.scalar.copy(condT[:], tp[:, 0:P])

        # gxs = cond @ Wxa[:COND] + ba  -> [P, 1152], then cast to bf16
        for sl in ((0, 512), (512, 1024), (1024, G3)):
            gsps = ps_l.tile([P, 2 * K], f32, tag="lps")
            nc.tensor.matmul(gsps[:, :sl[1] - sl[0]], condT[:], wxac[:, sl[0]:sl[1]],
                             start=True, stop=False)
            nc.tensor.matmul(gsps[:, :sl[1] - sl[0]], ones1[:], ba[:, sl[0]:sl[1]],
                             start=False, stop=True)
            nc.vector.tensor_copy(gxs[:, sl[0]:sl[1]], gsps[:, :sl[1] - sl[0]])
            nc.vector.tensor_copy(gxsr[:, sl[0]:sl[1]], gsps[:, :sl[1] - sl[0]])

        # ---- prologue: p(0) path + open accumulation groups for t=0 ----
        pdot = wk.tile([P, 1], f32, tag="pdot")
        sc16 = wk.tile([P, 16], f32, tag="sc16")
        nc.vector.scalar_tensor_tensor(sc16[:], lpcrot[:, 0:16], 0.0, ring[:],
                                       op0=ALU.bypass, op1=ALU.mult,
                                       accum_out=pdot[:])
        pscr = wk.tile([P, K], f32, tag="pscr")
        nc.vector.tensor_scalar(pscr[:], taub[:], pdot[:], 0.0,
                                op0=ALU.is_le, op1=ALU.add,
                                accum_out=dynstage[:, 0:1])

        def open_groups(dump=False):
            """Emit the t+1 accumulations that depend only on haT/hbTa/statics."""
            garz = ps_rz.tile([P, 768], f32, tag="garz")
            psn = ps_n.tile([P, HA + HB], f32, tag="ghn")
            ghn = psn[:, 0:HA]
            ghbn = psn[:, HA:HA + HB]
            gxn = ps_x.tile([P, HA], f32, tag="gxn")
            gb = ps_b.tile([P, 48], f32, tag="gb")
            # rz: gxs + sum_c haT_c @ Wha_c[rz]   (dyn closes later)
            # (single-matmul output is capped at one PSUM bank: 512 fp32)
            nc.tensor.matmul(garz[:, 0:512], identr[:], gxsr[:, 0:512],
                             start=True, stop=False)
            nc.tensor.matmul(garz[:, 512:768], identr[:], gxsr[:, 512:768],
                             start=True, stop=False)
            for c in range(3):
                w0 = c * G3
                nc.tensor.matmul(garz[:, 0:512], haT[c], wha[:, w0:w0 + 512],
                                 start=False, stop=False)
                nc.tensor.matmul(garz[:, 512:768], haT[c],
                                 wha[:, w0 + 512:w0 + 768],
                                 start=False, stop=False)
            # n (h-part): sum_c haT_c @ Wha_c[n]  (closed here)
            for c in range(3):
                w0 = c * G3
                nc.tensor.matmul(ghn[:], haT[c], wha[:, w0 + 768:w0 + G3],
                                 start=(c == 0), stop=(c == 2))
            # n (x-part): gxs_n  (dyn closes later)
            nc.tensor.matmul(gxn[:], identr[:], gxsr[:, 768:G3],
                             start=True, stop=False)
            # GRU-B: biases folded into the hbTa matmul (ones row of hbTa).
            # One accumulation group per PSUM bank: start=True clears the
            # whole bank's has_written bits, so ghb_n lives in the ps_n bank
            # (whose groups are emitted before it) and gb holds one group.
            nc.tensor.matmul(gb[:, 0:48], hbTa[:], whbbx[:, 0:48],
                             start=True, stop=False)
            nc.tensor.matmul(ghbn[:], hbTa[:], whbbx[:, 48:64],
                             start=True, stop=True)
            if dump:
                nc.sync.dma_start(dbg_outs["d_hbTa1"][:], hbTa[:])
                gbo_c = wk.tile([P, 64], f32, tag="dbg_gbo")
                nc.vector.tensor_copy(gbo_c[:, 0:48], gb[:])
                nc.vector.tensor_copy(gbo_c[:, 48:64], ghbn[:])
                nc.sync.dma_start(dbg_outs["d_gb1o"][:], gbo_c[:])
            return garz, ghn, ghbn, gxn, gb

        garz, ghn, ghbn, gxn, gb = open_groups()

        ha_rm = wk.tile([P, HA], f32, tag="ha_rm")
        nc.vector.memset(ha_rm[:], 0.0)
        hb_rm = wk.tile([P, HB], f32, tag="hb_rm")
        nc.vector.memset(hb_rm[:], 0.0)

        # ---- time loop ----
        for t in range(T_steps):
            # gumbel prefetch
            a2 = io.tile([P, K], f32, tag="a2")
            nc.sync.dma_start(a2[:], ins["gl"][t, :, :])

            # dyn features -> transposed [3, P]
            tpd = ps_t.tile([P, 512], f32, tag="tpw")
            nc.tensor.transpose(tpd[:3, 384:512], dynstage[:], ident[:])
            dynT = wk.tile([3, P], bf16, tag="dynT")
            nc.vector.tensor_copy(dynT[:], tpd[:3, 384:512])
            if dbg and t == 1:
                dynTf = wk.tile([3, P], f32, tag="dynTf")
                nc.vector.tensor_copy(dynTf[:], dynT[:])
                nc.sync.dma_start(dbg_outs["d_dynT1"][:], dynTf[:])

            # dyn matmuls close the rz / gxn groups (exact bf16 hi/lo)
            nc.tensor.matmul(garz[:, 0:512], dynT[:], wdynh[:, 0:512],
                             start=False, stop=False)
            nc.tensor.matmul(garz[:, 0:512], dynT[:], wdynl[:, 0:512],
                             start=False, stop=True)
            nc.tensor.matmul(garz[:, 512:768], dynT[:], wdynh[:, 512:768],
                             start=False, stop=False)
            nc.tensor.matmul(garz[:, 512:768], dynT[:], wdynl[:, 512:768],
                             start=False, stop=True)
            nc.tensor.matmul(gxn[:], dynT[:], wdynh[:, 768:G3],
                             start=False, stop=False)
            nc.tensor.matmul(gxn[:], dynT[:], wdynl[:, 768:G3],
                             start=False, stop=True)

            # GRU-A gates (read PSUM directly)
            r = wk.tile([P, HA], f32, tag="r")
            nc.scalar.activation(r[:], garz[:, 0:HA], AF.Sigmoid)
            z = wk.tile([P, HA], f32, tag="z")
            nc.scalar.activation(z[:], garz[:, HA:768], AF.Sigmoid)
            t1 = wk.tile([P, HA], f32, tag="t1")
            nc.vector.tensor_tensor(t1[:], r[:], ghn[:], op=ALU.mult)
            t3 = wk.tile([P, HA], f32, tag="t3")
            nc.vector.tensor_tensor(t3[:], t1[:], gxn[:], op=ALU.add)
            an = wk.tile([P, HA], f32, tag="an")
            nc.scalar.activation(an[:], t3[:], AF.Tanh)
            d = wk.tile([P, HA], f32, tag="d")
            nc.vector.tensor_tensor(d[:], ha_rm[:], an[:], op=ALU.subtract)
            zd = wk.tile([P, HA], f32, tag="zd")
            nc.vector.tensor_tensor(zd[:], d[:], z[:], op=ALU.mult)
            ha_rm = wk.tile([P, HA], f32, tag="ha_rm")
            nc.vector.tensor_tensor(ha_rm[:], zd[:], an[:], op=ALU.add)
            if dbg and t == 0:
                nc.sync.dma_start(dbg_outs["d_ha0"][:], ha_rm[:])
            if dbg and t == 1:
                garz_c = wk.tile([P, 768], f32, tag="dbg_garz")
                nc.vector.tensor_copy(garz_c[:], garz[:])
                nc.sync.dma_start(dbg_outs["d_garz1"][:], garz_c[:])
                ghn_c = wk.tile([P, HA], f32, tag="dbg_ghn")
                nc.vector.tensor_copy(ghn_c[:], ghn[:])
                nc.sync.dma_start(dbg_outs["d_ghn1"][:], ghn_c[:])
                gxn_c = wk.tile([P, HA], f32, tag="dbg_gxn")
                nc.vector.tensor_copy(gxn_c[:], gxn[:])
                nc.sync.dma_start(dbg_outs["d_gxn1"][:], gxn_c[:])
                nc.sync.dma_start(dbg_outs["d_r1"][:], r[:])
                nc.sync.dma_start(dbg_outs["d_an1"][:], an[:])
                nc.sync.dma_start(dbg_outs["d_ha1"][:], ha_rm[:])

            # ha2 -> haT (3 chunk transposes; copies on 3 engines)
            tpc = ps_t.tile([P, 512], f32, tag="tpw")
            for c in range(3):
                nc.tensor.transpose(tpc[:, c * P:(c + 1) * P],
                                    ha_rm[:, c * P:(c + 1) * P], ident[:])
            nc.scalar.copy(haTw[:, 0:P], tpc[:, 0:P])
            nc.vector.tensor_copy(haTw[:, P:2 * P], tpc[:, P:2 * P])
            nc.scalar.copy(haTw[:, 2 * P:3 * P], tpc[:, 2 * P:3 * P])

            # GRU-B x-matmuls close the gb group (rz+n contiguous)
            for c in range(3):
                nc.tensor.matmul(gb[:, 0:48], haT[c], wxb[:, c * 48:(c + 1) * 48],
                                 start=False, stop=(c == 2))

            # GRU-B gates
            rzb = wk.tile([P, 2 * HB], f32, tag="rzb")
            nc.scalar.activation(rzb[:], gb[:, 0:32], AF.Sigmoid)
            t1b = wk.tile([P, HB], f32, tag="t1b")
            nc.vector.tensor_tensor(t1b[:], rzb[:, 0:HB], ghbn[:], op=ALU.mult)
            t2b = wk.tile([P, HB], f32, tag="t2b")
            nc.vector.tensor_tensor(t2b[:], t1b[:], gb[:, 32:48], op=ALU.add)
            nb = wk.tile([P, HB], f32, tag="nb")
            nc.scalar.activation(nb[:], t2b[:], AF.Tanh)
            db = wk.tile([P, HB], f32, tag="db")
            nc.vector.tensor_tensor(db[:], hb_rm[:], nb[:], op=ALU.subtract)
            zdb = wk.tile([P, HB], f32, tag="zdb")
            nc.vector.tensor_tensor(zdb[:], db[:], rzb[:, HB:2 * HB], op=ALU.mult)
            hb_rm = wk.tile([P, HB], f32, tag="hb_rm")
            nc.vector.tensor_tensor(hb_rm[:], zdb[:], nb[:], op=ALU.add)
            if dbg and t == 0:
                nc.sync.dma_start(dbg_outs["d_hb0"][:], hb_rm[:])
            if dbg and t == 1:
                gb_c = wk.tile([P, 64], f32, tag="dbg_gb")
                nc.vector.tensor_copy(gb_c[:, 0:48], gb[:])
                nc.vector.tensor_copy(gb_c[:, 48:64], ghbn[:])
                nc.sync.dma_start(dbg_outs["d_gb1"][:], gb_c[:])
                nc.sync.dma_start(dbg_outs["d_hb1"][:], hb_rm[:])

            # hb -> hbT, logits
            tpb = ps_t.tile([P, 512], f32, tag="tpw")
            nc.tensor.transpose(tpb[:HB, 384:512], hb_rm[:], ident[:])
            nc.scalar.copy(hbTa[:HB, :], tpb[:HB, 384:512])

            lps = ps_l.tile([P, 2 * K], f32, tag="lps")
            nc.tensor.matmul(lps[:], hbTa[:], w12b[:], start=True, stop=True)
            l12 = wk.tile([P, 2 * K], f32, tag="l12")
            nc.scalar.activation(l12[:], lps[:], AF.Tanh)
            lg12 = wk.tile([P, 2 * K], f32, tag="lg12")
            nc.vector.tensor_tensor(lg12[:], l12[:], g12[:], op=ALU.mult)
            logits = io.tile([P, K], f32, tag="logits")
            nc.vector.tensor_tensor(logits[:], lg12[:, 0:K], lg12[:, K:2 * K],
                                    op=ALU.add)
            nc.sync.dma_start(out_l[t, :, :], logits[:])

            # gumbel softmax expected index
            zz = wk.tile([P, K], f32, tag="zz")
            nc.vector.tensor_tensor(zz[:], logits[:], a2[:], op=ALU.subtract)
            E = wk.tile([P, K], f32, tag="E")
            den = wk.tile([P, 1], f32, tag="den")
            nc.scalar.activation(E[:], zz[:], AF.Exp, accum_out=den[:])
            Escr = wk.tile([P, K], f32, tag="Escr")
            num = wk.tile([P, 1], f32, tag="num")
            nc.vector.scalar_tensor_tensor(Escr[:], E[:], 0.0, idxb[:],
                                           op0=ALU.bypass, op1=ALU.mult,
                                           accum_out=num[:])
            rden = wk.tile([P, 1], f32, tag="rden")
            nc.vector.reciprocal(rden[:], den[:])
            soft = wk.tile([P, 1], f32, tag="soft")
            nc.vector.tensor_tensor(soft[:], num[:], rden[:], op=ALU.mult)
            if dbg and t == 0:
                nc.sync.dma_start(dbg_outs["d_soft0"][:], soft[:])
            # e = round-half-even(soft) == jnp.round (verified on HW)
            eint = wk.tile([P, 1], i32, tag="eint")
            nc.vector.tensor_copy(eint[:], soft[:])
            nc.vector.tensor_copy(dynstage[:, 2:3], eint[:])
            # s = p + e -> ring slot; col1 = p (before tau overwrites col0)
            nc.vector.tensor_tensor(ring[:, t % 16:t % 16 + 1], dynstage[:, 0:1],
                                    dynstage[:, 2:3], op=ALU.add)
            nc.vector.tensor_copy(dynstage[:, 1:2], dynstage[:, 0:1])
            # p(t+1) path
            rot = ((t + 1) % 16) * 16
            sc16 = wk.tile([P, 16], f32, tag="sc16")
            pdot = wk.tile([P, 1], f32, tag="pdot")
            nc.vector.scalar_tensor_tensor(sc16[:], lpcrot[:, rot:rot + 16], 0.0,
                                           ring[:], op0=ALU.bypass, op1=ALU.mult,
                                           accum_out=pdot[:])
            pscr = wk.tile([P, K], f32, tag="pscr")
            nc.vector.tensor_scalar(pscr[:], taub[:], pdot[:], 0.0,
                                    op0=ALU.is_le, op1=ALU.add,
                                    accum_out=dynstage[:, 0:1])
            if dbg and t == 0:
                nc.sync.dma_start(dbg_outs["d_e0"][:], dynstage[:, 2:3])
                nc.sync.dma_start(dbg_outs["d_s0"][:], ring[:, 0:1])
                nc.sync.dma_start(dbg_outs["d_p0"][:], dynstage[:, 1:2])
                nc.sync.dma_start(dbg_outs["d_p1"][:], dynstage[:, 0:1])
                nc.sync.dma_start(dbg_outs["d_pdot1"][:], pdot[:])

            # open accumulation groups for t+1 (hidden under this step's tail)
            if t + 1 < T_steps:
                garz, ghn, ghbn, gxn, gb = open_groups(dump=(dbg and t == 0))

    return nc, ins, out_l


def _pack_inputs(frames_features, lpc_coeffs, gumbel_u, Wf1, bf1, Wf2, bf2,
                 Wxa, Wha, ba, Wxb, Whb, bb, W1, b1, g1, W2, b2, g2,
                 T_steps=T):
    """Host-side packing -> list of per-core input dicts."""
    import ml_dtypes
    fp = np.float32
    bf = ml_dtypes.bfloat16
    feat = np.ascontiguousarray(frames_features, fp).reshape(R, NF)
    lpc = np.ascontiguousarray(lpc_coeffs, fp).reshape(R, M)
    u = np.ascontiguousarray(gumbel_u, fp)
    gl = np.log(-np.log(u[:T_steps])).astype(fp)

    # lpcrot[:, 16*r + j] = lpc[:, (j - r) % 16]
    lpcrot = np.empty((R, 256), fp)
    for r_ in range(16):
        for j in range(16):
            lpcrot[:, 16 * r_ + j] = lpc[:, (j - r_) % 16]

    wha_p = np.concatenate([np.ascontiguousarray(Wha[c * P:(c + 1) * P, :], fp)
                            for c in range(3)], axis=1)          # [128, 3*1152]
    # wxb chunk c: [rz 32 | n 16]
    wxb_p = np.concatenate(
        [np.concatenate([np.asarray(Wxb, fp)[c * P:(c + 1) * P, 0:32],
                         np.asarray(Wxb, fp)[c * P:(c + 1) * P, 32:48]], axis=1)
         for c in range(3)], axis=1)                             # [128, 144]
    statics = {
        "wf1b": np.concatenate([np.asarray(Wf1, fp), np.asarray(bf1, fp)[None, :]], 0),
        "wf2": np.asarray(Wf2, fp),
        "bf2": np.asarray(bf2, fp)[None, :],
        "wxac": np.ascontiguousarray(np.asarray(Wxa, fp)[:COND, :]),
        "ba": np.asarray(ba, fp)[None, :],
        "wdynh": _wdyn2(np.asarray(Wxa, fp))[0:3].astype(bf),
        "wdynl": _wdyn2(np.asarray(Wxa, fp))[3:6].astype(bf),
        "wha": wha_p,
        "wxb": wxb_p,
        "whbbx": np.concatenate([
            np.concatenate([np.asarray(Whb, fp)[:, 0:32],
                            np.asarray(bb, fp)[None, 0:32]], 0),
            np.concatenate([np.zeros((HB, HB), fp),
                            np.asarray(bb, fp)[None, 32:48]], 0),
            np.concatenate([np.asarray(Whb, fp)[:, 32:48],
                            np.zeros((1, HB), fp)], 0)], axis=1),
        "w12b": np.concatenate([
            np.concatenate([np.asarray(W1, fp), np.asarray(W2, fp)], axis=1),
            np.concatenate([np.asarray(b1, fp), np.asarray(b2, fp)])[None, :]],
            0),
        "g12": np.repeat(np.concatenate([np.asarray(g1, fp), np.asarray(g2, fp)])[None, :], P, 0),
        "idxb": np.repeat(np.arange(K, dtype=fp)[None, :], P, 0),
        "ident": np.eye(P, dtype=fp),
        "identr": np.eye(P, dtype=fp),
        "onesrow": np.ones((1, P), fp),
        "onesrowr": np.ones((1, P), fp),
        "zhaT": np.zeros((P, 3 * P), fp),
        "zhbT": np.zeros((HB, P), fp),
        "taub": np.repeat(_tau_table()[None, :], P, 0),
    }
    per_core = []
    for c in range(N_CORES):
        rs = slice(c * P, (c + 1) * P)
        m = dict(statics)
        m["feat"] = np.ascontiguousarray(feat[rs])
        m["lpcrot"] = np.ascontiguousarray(lpcrot[rs])
        m["gl"] = np.ascontiguousarray(gl[:, rs, :])
        per_core.append(m)
    return per_core


def _wdyn2(Wxa):
    """[6, 3H]: bf16 hi/lo split of [w_p, w_s, w_s+w_e] (exact-bf16 dyn matmul)."""
    import ml_dtypes
    fp = np.float32
    wd = Wxa[COND:COND + 3, :].astype(fp).copy()
    wd[2] = (wd[1] + wd[2]).astype(fp)
    hi = wd.astype(ml_dtypes.bfloat16).astype(fp)
    lo = (wd - hi).astype(ml_dtypes.bfloat16).astype(fp)
    return np.concatenate([hi, lo], 0)


def _tau_table():
    """tau[k] = smallest float32 x with mu_law_p(x) >= k+1 (k=0..254);
    tau[255] = +inf sentinel. p(x) = sum_k [x >= tau_k]."""
    fp = np.float32

    def p_of(x):
        x = np.asarray(x, fp)
        xc = np.clip(x, fp(-1.0), fp(1.0)).astype(fp)
        ln_mu1 = np.log(fp(256.0)).astype(fp)
        y = (np.sign(xc) * np.log1p(fp(255.0) * np.abs(xc)) / ln_mu1).astype(fp)
        v = ((y + fp(1.0)) * fp(0.5) * fp(256.0)).astype(fp)
        return np.clip(np.floor(v), 0.0, 255.0)

    def f2i(x):
        b = np.asarray(x, np.float32).view(np.int32)
        return np.where(b < 0, np.int32(-2147483648) - b, b).astype(np.int64)

    def i2f(i):
        i = np.asarray(i, np.int64)
        b = np.where(i < 0, -2147483648 - i, i).astype(np.int32)
        return b.view(np.float32)

    ks = np.arange(1, 256)
    lo = np.full(255, f2i(np.float32(-1.5)), np.int64)
    hi = np.full(255, f2i(np.float32(1.5)), np.int64)
    for _ in range(40):
        mid = (lo + hi) // 2
        ge = p_of(i2f(mid)) >= ks
        hi = np.where(ge, mid, hi)
        lo = np.where(ge, lo, mid)
    tau = i2f(hi).astype(fp)
    out = np.empty(256, fp)
    out[:255] = tau
    out[255] = np.float32(3.0e38)
    return out


_CACHE = {}


def _ensure_devices():
    import jax
    try:
        if len(jax.devices()) >= N_CORES:
            return
    except Exception:
        pass
    jax.config.update("jax_platforms", "axon,cpu")
    import jax.extend.backend as _jeb
    _jeb.clear_backends()
    assert len(jax.devices()) >= N_CORES, (
        f"need {N_CORES} NeuronCores, visible: {jax.devices()}")


def _get_nc(T_steps):
    if T_steps not in _CACHE:
        nc, ins, out_l = _build(T_steps)
        nc.compile()
        _CACHE[T_steps] = nc
    return _CACHE[T_steps]


def kernel(**inputs):
    _ensure_devices()
    nc = _get_nc(T)
    per_core = _pack_inputs(**inputs)
    res = run_bass_kernel_spmd(nc, per_core, list(range(N_CORES)))
    shards = [res.results[c]["logits"] for c in range(N_CORES)]   # each [T,128,K]
    logits_seq = np.concatenate(shards, axis=1)                   # [T, R, K]
    out = logits_seq.transpose(1, 0, 2).reshape(B, F * T, K)
    return np.ascontiguousarray(out, dtype=np.float32)
